# revision 2
# baseline (speedup 1.0000x reference)
"""Compressed multi-head attention (H=1) TRN2 Bass kernel, v3: fp16 scores path + fp8 hi/lo DoubleRow M/V phases.

Algebra (B=4, S=4096, E=D=1024, CF=4, Sc=1024):
    qkv = x @ w_qkv.T + b_qkv ; kc = conv_s4(k)+bkc ; vc = conv_s4(v)+bvc
    scores = q @ kc.T/sqrt(D) (+tril mask) ; y = softmax(scores) @ vc @ w_out.T

Device-side folds (all host-precomputed weights, fp16 on device):
    xw[c*E+e, j] = x[4j+c, e]                      # [4E, Sc] windowed x
    WMk = wq @ Wk_fold.T * (64/sqrt(D))            # [E, 4E]
    Mk  = WMk @ xw                                 # [E, Sc]  (device)
    scoresT*64 = Mk.T-contract xqT                 # [Sc, Sq] (device)
    exp with scale=1/64, bias=sbias (host: bq-dependent, else 0)
    WvE = Wv_fold @ w_out.T * 64                   # [4E, D]
    vc  = xw.T @ WvE (+64*bvc_eff)                 # [Sc, D]  (device)
    y   = (attn @ vc) / (attn . ones*64)           # 64s cancel
The per-q-row exp(q . bkc_eff) factor cancels in softmax normalization.

Sharding: 8 cores = 4 batches x 2 row-halves of S (2048 q rows/core).
Mk/vc are duplicated across the pair; scores/attention split by rows.

PE work per core: 512 (Mk) + 512 (vc) + 256 (scores) + 256 (attn@v)
[128x128x512] fp16 matmuls = 786k cycles ~ 328 us floor.
"""

import math
from contextlib import ExitStack

import numpy as np

B, S, E, D, CF = 4, 4096, 1024, 1024, 4
SC = S // CF            # 1024 compressed tokens
SQ = S // 2             # 2048 q rows per core
P = 128
NCORES = 8
KT = CF * E // P        # 32 contraction tiles over k' = 4E
ET = E // P             # 8 e tiles
CT = SC // P            # 8 compressed-token tiles
NG = SQ // 512          # 4 q groups of 512
SCALE = 64.0            # fp16 range scale folded into WMk/WvE
WS = 16.0               # extra x16 on fp8-hi weights, cancels via ones/exp scale

_prog_cache = {}


def _build_program(mask_active, add_fvec, add_vbias2):
    import concourse.bacc as bacc
    import concourse.mybir as mybir
    import concourse.tile as tile

    F32 = mybir.dt.float32
    F16 = mybir.dt.float16
    F8H = mybir.dt.float8e4
    F8L = mybir.dt.float8e5
    DR = mybir.MatmulPerfMode.DoubleRow

    nc = bacc.Bacc("TRN2")

    xwh = nc.dram_tensor("xwh", [CF * E, SC], F8H, kind="ExternalInput")
    xwl = nc.dram_tensor("xwl", [CF * E, SC], F8L, kind="ExternalInput")
    xqT = nc.dram_tensor("xqT", [E, SQ], F16, kind="ExternalInput")
    WMkh = nc.dram_tensor("WMkh", [CF * E // 2, 2, D], F8H,
                          kind="ExternalInput")
    WMkl = nc.dram_tensor("WMkl", [CF * E // 2, 2, D], F8L,
                          kind="ExternalInput")
    WvEh = nc.dram_tensor("WvEh", [CF * E // 2, 2, D], F8H,
                          kind="ExternalInput")
    WvEl = nc.dram_tensor("WvEl", [CF * E // 2, 2, D], F8L,
                          kind="ExternalInput")
    sbias = nc.dram_tensor("sbias", [P, CT], F32, kind="ExternalInput")
    maskM = None
    if mask_active:
        maskM = nc.dram_tensor("maskM", [SC, SQ // 2], F16, kind="ExternalInput")
    fvec = None
    if add_fvec:
        fvec = nc.dram_tensor("fvec", [P, D], F32, kind="ExternalInput")
    vb2 = None
    if add_vbias2:
        vb2 = nc.dram_tensor("vb2", [P, D], F32, kind="ExternalInput")
    y = nc.dram_tensor("y", [SQ, D], F16, kind="ExternalOutput")

    with tile.TileContext(nc) as tc, ExitStack() as top:
        persist = top.enter_context(tc.tile_pool(name="persist", bufs=1))
        xh_sb = persist.tile([P, KT, SC], F8H, tag="xwh")
        xl_sb = persist.tile([P, KT, SC], F8L, tag="xwl")
        Mk_sb = persist.tile([P, ET, SC], F16, tag="Mk")
        vc_sb = persist.tile([P, CT, D], F16, tag="vc")
        xq_sb = persist.tile([P, ET, SQ], F16, tag="xq")
        sb_sb = persist.tile([P, CT], F32, tag="sbias")
        ones16 = persist.tile([P, 2], F16, tag="ones16")
        nc.vector.memset(ones16, SCALE * WS)
        nc.sync.dma_start(out=sb_sb, in_=sbias[:])
        for kt in range(KT):
            nc.sync.dma_start(out=xh_sb[:, kt, :],
                              in_=xwh[kt * P:(kt + 1) * P, :])
            nc.sync.dma_start(out=xl_sb[:, kt, :],
                              in_=xwl[kt * P:(kt + 1) * P, :])
        for et in range(ET):
            nc.sync.dma_start(out=xq_sb[:, et, :],
                              in_=xqT[et * P:(et + 1) * P, :])
        fvec_sb = None
        if add_fvec:
            fvec_sb = persist.tile([P, D], F32, tag="fvec")
            nc.sync.dma_start(out=fvec_sb, in_=fvec[:])
        vb2_sb = None
        if add_vbias2:
            vb2_sb = persist.tile([P, D], F32, tag="vb2")
            nc.sync.dma_start(out=vb2_sb, in_=vb2[:])

        # ---------------- phase M: Mk = WMk @ xw (fp8 hi/lo DoubleRow) ----
        # W ~ hi(e4m3, x16) + lo(e5m2); x ~ hi(e4m3) + lo(e5m2).
        # psum = Wh.x_h + Wh.x_l + Wl.x_h  (lo*lo dropped), all at 16x scale
        # absorbed downstream (exp scale, ones value).
        # 4 half-set passes (scc x e-half) so the psum->sbuf copies of one
        # pass overlap the next pass's accumulation.
        KP = KT // 2            # 16 DoubleRow k'-pair steps
        with ExitStack() as ph:
            ws = ph.enter_context(tc.tile_pool(name="wm", bufs=4))
            pp = ph.enter_context(tc.tile_pool(name="pm", bufs=8, space="PSUM"))
            for scc in range(2):
                for eh in range(2):
                    pcs = [pp.tile([P, 512], F32, tag="mm", name=f"pm{e}")
                           for e in range(4)]
                    for kp in range(KP):
                        wh_sl = ws.tile([P, 2, 512], F8H, tag="wh")
                        wl_sl = ws.tile([P, 2, 512], F8L, tag="wl")
                        r0 = kp * P
                        cs = slice(eh * 512, (eh + 1) * 512)
                        nc.sync.dma_start(out=wh_sl,
                                          in_=WMkh[r0:r0 + P, :, cs])
                        nc.sync.dma_start(out=wl_sl,
                                          in_=WMkl[r0:r0 + P, :, cs])
                        kpr = slice(kp * 2, kp * 2 + 2)
                        scs = slice(scc * 512, (scc + 1) * 512)
                        for e in range(4):
                            es = slice(e * P, (e + 1) * P)
                            nc.tensor.matmul(
                                pcs[e], wh_sl[:, :, es], xh_sb[:, kpr, scs],
                                start=(kp == 0), stop=False, perf_mode=DR)
                            nc.tensor.matmul(
                                pcs[e], wh_sl[:, :, es], xl_sb[:, kpr, scs],
                                start=False, stop=False, perf_mode=DR)
                            nc.tensor.matmul(
                                pcs[e], wl_sl[:, :, es], xh_sb[:, kpr, scs],
                                start=False, stop=(kp == KP - 1), perf_mode=DR)
                    for e in range(4):
                        nc.vector.tensor_copy(
                            out=Mk_sb[:, eh * 4 + e,
                                      scc * 512:(scc + 1) * 512],
                            in_=pcs[e])

        # ---------------- phase V: vc = xw.T @ WvE (fp8 hi/lo DoubleRow) --
        with ExitStack() as ph:
            ws = ph.enter_context(tc.tile_pool(name="wv", bufs=4))
            pp = ph.enter_context(tc.tile_pool(name="pv", bufs=8, space="PSUM"))
            for oc in range(2):
                for jh in range(2):
                    pvs = [pp.tile([P, 512], F32, tag="mm", name=f"pv{j}")
                           for j in range(4)]
                    for kp in range(KP):
                        wh_sl = ws.tile([P, 2, 512], F8H, tag="wh")
                        wl_sl = ws.tile([P, 2, 512], F8L, tag="wl")
                        r0 = kp * P
                        cs = slice(oc * 512, (oc + 1) * 512)
                        nc.sync.dma_start(out=wh_sl,
                                          in_=WvEh[r0:r0 + P, :, cs])
                        nc.sync.dma_start(out=wl_sl,
                                          in_=WvEl[r0:r0 + P, :, cs])
                        kpr = slice(kp * 2, kp * 2 + 2)
                        for j in range(4):
                            jt = jh * 4 + j
                            js = slice(jt * P, (jt + 1) * P)
                            nc.tensor.matmul(
                                pvs[j], xh_sb[:, kpr, js], wh_sl,
                                start=(kp == 0), stop=False, perf_mode=DR)
                            nc.tensor.matmul(
                                pvs[j], xl_sb[:, kpr, js], wh_sl,
                                start=False, stop=False, perf_mode=DR)
                            nc.tensor.matmul(
                                pvs[j], xh_sb[:, kpr, js], wl_sl,
                                start=False, stop=(kp == KP - 1), perf_mode=DR)
                    for j in range(4):
                        dst = vc_sb[:, jh * 4 + j, oc * 512:(oc + 1) * 512]
                        if add_vbias2:
                            nc.vector.tensor_tensor(
                                out=dst, in0=pvs[j],
                                in1=vb2_sb[:, oc * 512:(oc + 1) * 512],
                                op=mybir.AluOpType.add)
                        else:
                            nc.vector.tensor_copy(out=dst, in_=pvs[j])

        # ---------------- phase D: scores + softmax + attn@v ----------------
        with ExitStack() as ph:
            mk = None
            if mask_active:
                mkp = ph.enter_context(tc.tile_pool(name="mkp", bufs=1))
                mk = mkp.tile([P, CT, SQ // 2], F16)
                for ct in range(CT):
                    nc.sync.dma_start(out=mk[:, ct, :],
                                      in_=maskM[ct * P:(ct + 1) * P, :])
            att = ph.enter_context(tc.tile_pool(name="att", bufs=2))
            yp = ph.enter_context(tc.tile_pool(name="yp", bufs=3))
            rp = ph.enter_context(tc.tile_pool(name="rp", bufs=4))
            pS = ph.enter_context(tc.tile_pool(name="pS", bufs=2, space="PSUM"))
            pD = ph.enter_context(tc.tile_pool(name="pD", bufs=2, space="PSUM"))
            def emit_scores(g):
                q0 = g * 512
                at = att.tile([P, CT, 512], F16, tag="at", name=f"at{g % 2}")
                for jt in range(CT):
                    sc = pS.tile([P, 512], F32, tag="sc")
                    for e in range(ET):
                        nc.tensor.matmul(
                            sc,
                            Mk_sb[:, e, jt * P:(jt + 1) * P],
                            xq_sb[:, e, q0:q0 + 512],
                            start=(e == 0), stop=(e == ET - 1))
                    nc.scalar.activation(
                        out=at[:, jt, :], in_=sc,
                        func=mybir.ActivationFunctionType.Exp,
                        scale=1.0 / (SCALE * WS), bias=sb_sb[:, jt:jt + 1])
                    if mask_active and g < 2:
                        nc.vector.tensor_tensor(
                            out=at[:, jt, :], in0=at[:, jt, :],
                            in1=mk[:, jt, q0:q0 + 512],
                            op=mybir.AluOpType.mult)
                return at

            def emit_attn(g, at):
                q0 = g * 512
                for qp in range(4):
                    po = pD.tile([P, D], F32, tag="out")
                    psm = pD.tile([P, 2], F32, tag="sums")
                    for ct in range(CT):
                        lhsT = at[:, ct, qp * P:(qp + 1) * P]
                        nc.tensor.matmul(
                            po[:, 0:512], lhsT, vc_sb[:, ct, 0:512],
                            start=(ct == 0), stop=(ct == CT - 1))
                        nc.tensor.matmul(
                            po[:, 512:1024], lhsT, vc_sb[:, ct, 512:1024],
                            start=(ct == 0), stop=(ct == CT - 1))
                        nc.tensor.matmul(
                            psm, lhsT, ones16,
                            start=(ct == 0), stop=(ct == CT - 1))
                    rinv = rp.tile([P, 1], F32, tag="rinv")
                    nc.vector.reciprocal(out=rinv, in_=psm[:, 0:1])
                    y_sb = yp.tile([P, D], F16, tag="y")
                    if add_fvec:
                        y32 = yp.tile([P, D], F32, tag="y32")
                        nc.vector.tensor_scalar_mul(
                            out=y32, in0=po, scalar1=rinv)
                        nc.vector.tensor_tensor(
                            out=y_sb, in0=y32, in1=fvec_sb,
                            op=mybir.AluOpType.add)
                    else:
                        nc.vector.tensor_scalar_mul(
                            out=y_sb, in0=po, scalar1=rinv)
                    r0 = q0 + qp * P
                    nc.sync.dma_start(out=y[r0:r0 + P, :], in_=y_sb)

            prev = None
            for g in range(NG):
                at = emit_scores(g)
                if prev is not None:
                    emit_attn(g - 1, prev)
                prev = at
            emit_attn(NG - 1, prev)

    nc.compile()
    return nc


def _get_program(mask_active, add_fvec, add_vbias2):
    key = (mask_active, add_fvec, add_vbias2)
    if key not in _prog_cache:
        _prog_cache[key] = _build_program(*key)
    return _prog_cache[key]


def prepare(x, w_qkv, b_qkv, wk_conv, bk_conv, wv_conv, bv_conv, w_out, b_out,
            mask):
    x = np.asarray(x, np.float32)
    w_qkv = np.asarray(w_qkv, np.float32)
    b_qkv = np.asarray(b_qkv, np.float32)
    wk_conv = np.asarray(wk_conv, np.float32)
    bk_conv = np.asarray(bk_conv, np.float32)
    wv_conv = np.asarray(wv_conv, np.float32)
    bv_conv = np.asarray(bv_conv, np.float32)
    w_out = np.asarray(w_out, np.float32)
    b_out = np.asarray(b_out, np.float32)
    mask_active = bool(np.asarray(mask).reshape(-1)[0])

    wT = w_qkv.T                                   # [E, 3D]
    wq, wk, wv = wT[:, :D], wT[:, D:2 * D], wT[:, 2 * D:]
    # Wk_fold[c*E+e, d] = sum_dd wk[e,dd] * wk_conv[d,dd,c]
    Wk_fold = np.einsum('ed,odc->ceo', wk, wk_conv,
                        optimize=True).reshape(CF * E, D)
    Wv_fold = np.einsum('ed,odc->ceo', wv, wv_conv,
                        optimize=True).reshape(CF * E, D) @ w_out.T
    bq, bk_, bv_ = b_qkv[:D], b_qkv[D:2 * D], b_qkv[2 * D:]
    bkc_eff = wk_conv.sum(axis=2) @ bk_ + bk_conv
    bvc_eff = (wv_conv.sum(axis=2) @ bv_ + bv_conv) @ w_out.T

    import ml_dtypes
    E4, E5 = ml_dtypes.float8_e4m3, ml_dtypes.float8_e5m2

    def hilo(a):
        hi = np.ascontiguousarray(a).astype(E4)
        lo = np.ascontiguousarray(a - hi.astype(np.float32)).astype(E5)
        return hi, lo

    def pack_pairs(w):
        # [4096, 1024] -> [2048, 2, 1024]: row (2*kp+i)*128+p -> [kp*128+p, i]
        n = w.shape[0]
        return np.ascontiguousarray(
            w.reshape(n // 256, 2, P, -1).swapaxes(1, 2)
            .reshape(n // 2, 2, -1))

    WMkh16, WMkl16 = hilo((Wk_fold @ wq.T) * (WS * SCALE / math.sqrt(D)))
    WvEh16, WvEl16 = hilo(Wv_fold * (WS * SCALE))
    WMkh16, WMkl16 = pack_pairs(WMkh16), pack_pairs(WMkl16)
    WvEh16, WvEl16 = pack_pairs(WvEh16), pack_pairs(WvEl16)
    # score bias rows: sbias_j = (bq . Wk_fold.T@xw_j + bq . bkc_eff)/sqrt(D)
    wsb = (bq @ Wk_fold.T) / math.sqrt(D)          # [4E]
    sb_const = float(bq @ bkc_eff) / math.sqrt(D)
    b_vc2 = bvc_eff * SCALE * WS
    add_vbias2 = bool(np.any(bvc_eff))
    add_fvec = bool(np.any(b_out))

    nc = _get_program(mask_active, add_fvec, add_vbias2)

    xw_list, sb_list, xq_list = [], [], []
    for b in range(B):
        xb = x[b]
        xw_f32 = xb.reshape(SC, CF, E).transpose(1, 2, 0).reshape(CF * E, SC)
        xw_list.append(hilo(xw_f32))
        sb_v = (wsb @ xw_f32 + sb_const).astype(np.float32)    # [SC]
        sb_list.append(np.ascontiguousarray(sb_v.reshape(CT, P).T))
        xq_list.append(np.ascontiguousarray(xb.T.astype(np.float16)))

    if mask_active:
        mm_real = np.ascontiguousarray(
            (np.arange(SC)[:, None] <= np.arange(SQ // 2)[None, :])
            .astype(np.float16))
        mm_ones = np.ones((SC, SQ // 2), np.float16)

    in_maps = []
    for core in range(NCORES):
        b, h = divmod(core, 2)
        m = {
            "xwh": xw_list[b][0], "xwl": xw_list[b][1],
            "xqT": np.ascontiguousarray(xq_list[b][:, h * SQ:(h + 1) * SQ]),
            "WMkh": WMkh16, "WMkl": WMkl16,
            "WvEh": WvEh16, "WvEl": WvEl16,
            "sbias": sb_list[b],
        }
        if mask_active:
            m["maskM"] = mm_real if h == 0 else mm_ones
        if add_fvec:
            m["fvec"] = np.ascontiguousarray(
                np.broadcast_to(b_out[None, :], (P, D)).astype(np.float32))
        if add_vbias2:
            m["vb2"] = np.ascontiguousarray(
                np.broadcast_to(b_vc2[None, :], (P, D)).astype(np.float32))
        in_maps.append(m)
    return nc, in_maps


def assemble(results):
    out = np.empty((B, S, D), np.float32)
    for core in range(NCORES):
        b, h = divmod(core, 2)
        out[b, h * SQ:(h + 1) * SQ, :] = results[core]["y"].astype(np.float32)
    return out


def kernel(x, w_qkv, b_qkv, wk_conv, bk_conv, wv_conv, bv_conv, w_out, b_out,
           mask):
    from concourse.bass_utils import run_bass_kernel_spmd

    nc, in_maps = prepare(x, w_qkv, b_qkv, wk_conv, bk_conv, wv_conv, bv_conv,
                          w_out, b_out, mask)
    res = run_bass_kernel_spmd(nc, in_maps, core_ids=list(range(NCORES)))
    return assemble(res.results)
